# revision 30
# baseline (speedup 1.0000x reference)
"""Trainium2 Bass kernel for nn_DeChunkLayer.

Per batch row (one NeuronCore each, pure data parallel):
  1. gate[c]: boundary-sorted clipped probabilities (host, tiny).
  2. EMA linear recurrence over chunks h_c = (1-g_c) h_{c-1} + g_c x_c as a
     blocked lower-triangular matmul "scan": for each 128-chunk block t,
       ema_t = L_t @ X_t + L2_t @ X_{t-1}
     with coefficients host-computed in f64 log space. The one-block (128
     chunk) lookback is exact to fp16 resolution because the decay product
     over 128 chunks underflows far below fp32 (host-verified bound).
  3. Dechunk out[s] = ema[cid[s]] as one-hot selection matmuls per 128-token
     block. Each token block uses a single 128-chunk window of ema: either
     an aligned scan block, or a 64-offset window stitched from the two
     adjacent ema blocks by two cheap f16 partition copies. Token blocks
     whose cross-core union span exceeds any feasible window accumulate 2-3
     aligned-block matmuls in PSUM instead. Selection one-hot matrices are
     host-precomputed and DMAed (is_equal on-device costs DVE time that is
     needed for the PSUM->SBUF casts, which bound the kernel body).

All matmul operands are fp16 (PSUM accumulates fp32): values are O(5) so
fp16 keeps abs err ~4e-3 (rel ~3.5e-4) while running the PE at full rate.
"""

import math

import numpy as np

import concourse.bacc as bacc
import concourse.mybir as mybir
from concourse import tile
from concourse.bass_utils import run_bass_kernel_spmd

B, SEQ, MAXC, DIM = 8, 4096, 2048, 1024
BLK = 128
NTB = SEQ // BLK  # 32 token blocks
NCORES = 8
F32 = mybir.dt.float32
F16 = mybir.dt.float16
F8 = mybir.dt.float8e4
# output staging group sizes (token blocks per out DMA); small groups keep
# the out-DMA stream smooth
GRPS = [1, 1] + [2] * 15
assert sum(GRPS) == NTB


def _preprocess(chunk_states, boundary_mask, boundary_prob):
    """Host-side index/gate math.

    Returns (in_maps, NBLK, wins, prod_ws) where wins[tb] is the list of
    window starts for token block tb (64-aligned; singleton for the common
    stitched/direct case) and prod_ws the ordered list of 64-offset windows
    that must be stitched on device.
    """
    chunk_states = np.asarray(chunk_states, dtype=np.float32)
    boundary_mask = np.asarray(boundary_mask)
    boundary_prob = np.asarray(boundary_prob, dtype=np.float32)

    p_full = np.clip(boundary_prob[..., -1], np.float32(1e-4), np.float32(1.0 - 1e-4))
    token_idx = np.arange(SEQ)[None, :] + (~boundary_mask).astype(np.int32) * SEQ
    order = np.argsort(token_idx, axis=1, kind="stable")
    gate = np.take_along_axis(p_full, order[:, :MAXC], axis=1)  # [B, C]

    cid = np.cumsum(boundary_mask.astype(np.int32), axis=1) - 1  # [B, S]
    cid = np.clip(cid, 0, MAXC - 1)
    n_used = int(cid.max()) + 1
    NBLK = max(1, math.ceil(n_used / BLK))
    CU = NBLK * BLK

    g = gate[:, :CU].astype(np.float64)
    a = 1.0 - g
    S = np.cumsum(np.log(a), axis=1)  # [B, CU] global log-decay prefix

    # one-block lookback must cover everything older than the previous block
    for t in range(2, NBLK):
        j0 = (t - 1) * BLK - 1
        if np.any(S[:, t * BLK] - S[:, j0] > -18.0):
            raise RuntimeError("128-chunk lookback decay bound violated")

    ii = np.arange(BLK)[:, None]
    jj = np.arange(BLK)[None, :]
    Sb = S.reshape(B, NBLK, BLK)
    # main (within-block) coefficients: L[b,t,i,j] = g_j exp(S_i - S_j), i>=j
    Lf = np.where(
        ii[None, None] >= jj[None, None],
        np.exp(Sb[:, :, :, None] - Sb[:, :, None, :])
        * g.reshape(B, NBLK, 1, BLK),
        0.0,
    )
    # lhsT layout: lt[b, j, t*128 + i]
    LT_sb = np.ascontiguousarray(
        Lf.transpose(0, 3, 1, 2).reshape(B, BLK, NBLK * BLK).astype(np.float16)
    )

    # full-block lookback: chunk (t-1)*128+j feeding out chunk t*128+i
    lt2_sb = np.zeros((B, BLK, NBLK * BLK), dtype=np.float16)
    for t in range(1, NBLK):
        Sout = S[:, t * BLK:(t + 1) * BLK]  # [B, 128]
        Sin = S[:, (t - 1) * BLK:t * BLK]  # [B, 128]
        gin = g[:, (t - 1) * BLK:t * BLK]
        Lb = np.exp(Sout[:, None, :] - Sin[:, :, None]) * gin[:, :, None]
        lt2_sb[:, :, t * BLK:(t + 1) * BLK] = Lb.astype(np.float16)

    # dechunk windows: per token block one 64-aligned 128-chunk window when
    # the union (all cores) span allows it, else aligned 128-blocks.
    cidr = cid.reshape(B, NTB, BLK)
    lo = cidr[:, :, 0].min(axis=0)
    hi = cidr[:, :, -1].max(axis=0)
    wins = []
    chosen = set()
    for tb in range(NTB):
        l, h = int(lo[tb]), int(hi[tb])
        w_hi = (l // 64) * 64          # largest feasible start
        w_lo = max(0, -(-(h - 127) // 64) * 64)  # smallest feasible start
        pick = None
        if w_lo <= w_hi:
            cands = range(w_lo, w_hi + 1, 64)
            for w in cands:  # reuse an already-stitched window if possible
                if w in chosen:
                    pick = w
                    break
            if pick is None:
                for align in (128, 64):
                    for w in cands:
                        if w % align == 0:
                            pick = w
                            break
                    if pick is not None:
                        break
        if pick is None:
            t0, t1 = l // BLK, h // BLK
            wins.append([BLK * t for t in range(t0, t1 + 1)])
        else:
            chosen.add(pick)
            wins.append([pick])
    prod_ws = sorted({w for ws in wins for w in ws if w % BLK != 0})

    # host-precomputed one-hot selection matrices, in emit (tb, window)
    # column order: sel[p, j] = 1 iff cid[tb*128+j] == w + p
    ncols = sum(len(ws) for ws in wins)
    selh = np.zeros((B, BLK, ncols * BLK),
                    dtype=mybir.dt.np(mybir.dt.float8e4))
    col = 0
    for tb in range(NTB):
        for w in wins[tb]:
            c = cid[:, tb * BLK:(tb + 1) * BLK] - w  # [B, 128]
            m = (c >= 0) & (c < BLK)
            bb, jj2 = np.nonzero(m)
            selh[bb, c[bb, jj2], col * BLK + jj2] = 1.0
            col += 1

    in_maps = []
    for b in range(B):
        in_maps.append(
            {
                "x": np.ascontiguousarray(chunk_states[b, :CU].astype(np.float16)),
                "lt": LT_sb[b],
                "lt2": np.ascontiguousarray(lt2_sb[b]),
                "selh": selh[b],
            }
        )
    return in_maps, NBLK, wins, prod_ws


def _build_nc(NBLK, wins, prod_ws):
    ncols = sum(len(ws) for ws in wins)
    nc = bacc.Bacc("TRN2", target_bir_lowering=False, debug=False, num_devices=8)
    x = nc.dram_tensor("x", [NBLK * BLK, DIM], F16, kind="ExternalInput")
    lt = nc.dram_tensor("lt", [BLK, NBLK * BLK], F16, kind="ExternalInput")
    lt2 = nc.dram_tensor("lt2", [BLK, NBLK * BLK], F16, kind="ExternalInput")
    selh = nc.dram_tensor("selh", [BLK, ncols * BLK], F8, kind="ExternalInput")
    out = nc.dram_tensor("out", [SEQ, DIM], F16, kind="ExternalOutput")

    # per-window production step: w -> scan block whose CAST unblocks it
    def wneed(w):
        return w // BLK if w % BLK == 0 else min(w // BLK + 1, NBLK - 1)

    need_zblk = any(w // BLK == NBLK - 1 and w % BLK for w in prod_ws)
    prod_at = {}  # scan block t -> list of windows to stitch after its CAST
    for w in prod_ws:
        prod_at.setdefault(wneed(w), []).append(w)

    # sel DMA split points (in col units), aligned to tb boundaries so each
    # emit group's columns are covered by a whole DMA
    colofs = [0]
    for ws in wins:
        colofs.append(colofs[-1] + len(ws))  # colofs[tb] = first col of tb
    scuts = sorted({colofs[min(4, NTB)], colofs[min(12, NTB)],
                    colofs[min(22, NTB)], ncols})

    with tile.TileContext(nc) as tc:
        with (
            tc.tile_pool(name="const", bufs=1) as const_pool,
            tc.tile_pool(name="outp", bufs=6) as outpool,
            tc.tile_pool(name="ps_scan", bufs=1, space="PSUM") as ps_scan,
            tc.tile_pool(name="ps_out", bufs=3, space="PSUM") as ps_out,
        ):
            # load order: scan weights + x first (critical path) on the sync
            # ring; lookback weights + selection matrices on the scalar ring.
            lt_sb = const_pool.tile([BLK, NBLK * BLK], F16, tag="lt")
            nc.sync.dma_start(lt_sb[:], lt[:])
            x_sb = const_pool.tile([BLK, NBLK * DIM], F16, tag="x")
            # stage x in pieces so early scan blocks unblock fast
            xcuts = sorted({min(1, NBLK), min(2, NBLK), min(3, NBLK),
                            min(5, NBLK), min(7, NBLK), NBLK})
            c_prev = 0
            for c1 in xcuts:
                if c1 <= c_prev:
                    continue
                nc.sync.dma_start(
                    x_sb[:, c_prev * DIM:c1 * DIM],
                    x[c_prev * BLK:c1 * BLK, :].rearrange(
                        "(t p) d -> p t d", p=BLK),
                )
                c_prev = c1
            lt2_sb = const_pool.tile([BLK, NBLK * BLK], F16, tag="lt2")
            nc.scalar.dma_start(lt2_sb[:], lt2[:])
            sel_sb = const_pool.tile([BLK, ncols * BLK], F8, tag="selh")
            c_prev = 0
            for c1 in scuts:
                if c1 <= c_prev:
                    continue
                nc.scalar.dma_start(
                    sel_sb[:, c_prev * BLK:c1 * BLK],
                    selh[:, c_prev * BLK:c1 * BLK],
                )
                c_prev = c1

            # ema: one extra zeroed block so window stitches that read past
            # the last scan block pull zeros (sel never selects them)
            ema = const_pool.tile([BLK, (NBLK + 1) * DIM], F16, tag="ema")
            if need_zblk:
                nc.vector.memset(ema[:, NBLK * DIM:], 0.0)
            win_sb = {}
            for w in prod_ws:
                win_sb[w] = const_pool.tile([BLK, DIM], F16, tag=f"win{w}",
                                            name=f"win_{w}")

            # PE warmup: zero-weight matmuls accumulating into block 0's
            # psum (add 0, cannot be dead-code-eliminated). ~3us of PE
            # activity releases the HAM clock throttle before real work.
            zw = const_pool.tile([BLK, BLK], F16, tag="zw")
            nc.vector.memset(zw[:], 0.0)
            zx = const_pool.tile([BLK, 512], F16, tag="zx")
            nc.vector.memset(zx[:], 0.0)
            ps0 = ps_scan.tile([BLK, DIM], F32, tag="ps")
            for k in range(4):
                for h in range(2):
                    nc.tensor.matmul(
                        ps0[:, h * 512:(h + 1) * 512], lhsT=zw[:], rhs=zx[:],
                        start=(k == 0), stop=False,
                    )

            # ---- dechunk emitter (interleaved with the scan so the PE
            # queue never stalls behind scan blocks waiting on late DMAs) ---
            state = {"tb": 0, "gi": 0, "col": 0}

            def emit_group(grp):
                gi = state["gi"]
                og = outpool.tile([BLK, grp * DIM], F16, tag=f"og{grp}",
                                  name=f"og_{gi}")
                for i in range(grp):
                    tb = state["tb"]
                    ws = wins[tb]
                    po = ps_out.tile([BLK, DIM], F32, tag="po",
                                     name=f"po_{tb}")
                    for wi, w in enumerate(ws):
                        col = state["col"]
                        for h in range(2):
                            if w % BLK == 0:
                                rsrc = ema[:, (w // BLK) * DIM + h * 512:
                                           (w // BLK) * DIM + (h + 1) * 512]
                            else:
                                rsrc = win_sb[w][:, h * 512:(h + 1) * 512]
                            nc.tensor.matmul(
                                po[:, h * 512:(h + 1) * 512],
                                lhsT=sel_sb[:, col * BLK:(col + 1) * BLK],
                                rhs=rsrc,
                                start=(wi == 0),
                                stop=(wi == len(ws) - 1),
                            )
                        state["col"] = col + 1
                    dst = og[:, i * DIM:(i + 1) * DIM]
                    # split the PSUM->SBUF cast across both engines: halves
                    # run concurrently, so the po buffer frees quickly
                    nc.vector.tensor_copy(out=dst[:, :512], in_=po[:, :512])
                    nc.scalar.copy(out=dst[:, 512:], in_=po[:, 512:])
                    state["tb"] = tb + 1
                tb0 = state["tb"] - grp
                dma_eng = nc.scalar if (gi % 2) == 0 else nc.sync
                dma_eng.dma_start(
                    out[tb0 * BLK:state["tb"] * BLK, :].rearrange(
                        "(i p) d -> p i d", p=BLK
                    ),
                    og[:].rearrange("p (i d) -> p i d", d=DIM),
                )
                state["gi"] = gi + 1

            # a group is ready once the last scan block it depends on (via
            # direct windows or stitched ones) is written
            group_need = []
            tb = 0
            for grp in GRPS:
                group_need.append(
                    max(wneed(w) for t in range(tb, tb + grp)
                        for w in wins[t]))
                tb += grp

            # ---- blocked matmul scan over chunk blocks ----
            for t in range(NBLK):
                ps = ps0 if t == 0 else ps_scan.tile([BLK, DIM], F32, tag="ps")
                for h in range(2):
                    sl = slice(h * 512, (h + 1) * 512)
                    xsl = slice(t * DIM + h * 512, t * DIM + (h + 1) * 512)
                    nc.tensor.matmul(
                        ps[:, sl],
                        lhsT=lt_sb[:, t * BLK:(t + 1) * BLK],
                        rhs=x_sb[:, xsl],
                        start=(t != 0),
                        stop=(t == 0),
                    )
                    if t > 0:
                        lsl = slice((t - 1) * DIM + h * 512,
                                    (t - 1) * DIM + (h + 1) * 512)
                        nc.tensor.matmul(
                            ps[:, sl],
                            lhsT=lt2_sb[:, t * BLK:(t + 1) * BLK],
                            rhs=x_sb[:, lsl],
                            start=False,
                            stop=True,
                        )
                # psum -> fp16 ema. Early blocks split DVE/ACT for latency;
                # later blocks go to ACT, compensating for the stitch copies
                # that all run on DVE (DVE's 2x 16-bit mode does [64,1024]
                # f16 copies at 421ns vs 1145ns on ACT, which saturates)
                if t < 4:
                    nc.vector.tensor_copy(
                        out=ema[:, t * DIM:t * DIM + 512], in_=ps[:, :512]
                    )
                else:
                    nc.scalar.copy(
                        out=ema[:, t * DIM:t * DIM + 512], in_=ps[:, :512]
                    )
                nc.scalar.copy(
                    out=ema[:, t * DIM + 512:(t + 1) * DIM], in_=ps[:, 512:]
                )
                for w in prod_at.get(t, ()):
                    t0 = w // BLK
                    nc.vector.tensor_copy(
                        out=win_sb[w][0:64, :],
                        in_=ema[64:BLK, t0 * DIM:(t0 + 1) * DIM],
                    )
                    nc.vector.tensor_copy(
                        out=win_sb[w][64:BLK, :],
                        in_=ema[0:64, (t0 + 1) * DIM:(t0 + 2) * DIM],
                    )
                while (state["gi"] < len(GRPS)
                       and group_need[state["gi"]] <= t):
                    emit_group(GRPS[state["gi"]])

            while state["gi"] < len(GRPS):
                emit_group(GRPS[state["gi"]])

    nc.finalize()
    return nc


def _run(in_maps, NBLK, wins, prod_ws):
    nc = _build_nc(NBLK, wins, prod_ws)
    res = run_bass_kernel_spmd(nc, in_maps, core_ids=list(range(NCORES)))
    return np.stack(
        [res.results[i]["out"].astype(np.float32) for i in range(NCORES)], axis=0
    )


def kernel(chunk_states, boundary_mask, boundary_prob):
    in_maps, NBLK, wins, prod_ws = _preprocess(
        chunk_states, boundary_mask, boundary_prob
    )
    last_err = None
    for _ in range(3):  # retry transient accelerator failures
        try:
            return _run(in_maps, NBLK, wins, prod_ws)
        except Exception as e:  # noqa: BLE001
            last_err = e
            try:
                import jax

                jax.clear_caches()
            except Exception:  # noqa: BLE001
                pass
    raise last_err


# revision 32
# speedup vs baseline: 1.1547x; 1.1547x over previous
"""Trainium2 Bass kernel for nn_DeChunkLayer.

Per batch row (one NeuronCore each, pure data parallel):
  1. gate[c]: boundary-sorted clipped probabilities (host, tiny).
  2. EMA linear recurrence over chunks h_c = (1-g_c) h_{c-1} + g_c x_c as a
     blocked lower-triangular matmul "scan": for each 128-chunk block t,
       ema_t = L_t @ X_t + L2_t @ X_{t-1}
     with coefficients host-computed in f64 log space. The one-block (128
     chunk) lookback is exact to fp16 resolution because the decay product
     over 128 chunks underflows far below fp32 (host-verified bound).
  3. Dechunk out[s] = ema[cid[s]] as one-hot selection matmuls per 128-token
     block. Each token block uses a single 128-chunk window of ema: either
     an aligned scan block, or a 64-offset window stitched from the two
     adjacent ema blocks by two cheap f16 partition copies. Token blocks
     whose cross-core union span exceeds any feasible window accumulate 2-3
     aligned-block matmuls in PSUM instead. Selection one-hot matrices are
     host-precomputed and DMAed (is_equal on-device costs DVE time that is
     needed for the PSUM->SBUF casts, which bound the kernel body).

All matmul operands are fp16 (PSUM accumulates fp32): values are O(5) so
fp16 keeps abs err ~4e-3 (rel ~3.5e-4) while running the PE at full rate.
"""

import math

import numpy as np

import concourse.bacc as bacc
import concourse.mybir as mybir
from concourse import tile
from concourse.bass_utils import run_bass_kernel_spmd

B, SEQ, MAXC, DIM = 8, 4096, 2048, 1024
BLK = 128
NTB = SEQ // BLK  # 32 token blocks
NCORES = 8
F32 = mybir.dt.float32
F16 = mybir.dt.float16
F8 = mybir.dt.float8e4
# output staging group sizes (token blocks per out DMA); small groups keep
# the out-DMA stream smooth
GRPS = [1, 1] + [2] * 15
assert sum(GRPS) == NTB


def _preprocess(chunk_states, boundary_mask, boundary_prob):
    """Host-side index/gate math.

    Returns (in_maps, NBLK, wins, prod_ws) where wins[tb] is the list of
    window starts for token block tb (64-aligned; singleton for the common
    stitched/direct case) and prod_ws the ordered list of 64-offset windows
    that must be stitched on device.
    """
    chunk_states = np.asarray(chunk_states, dtype=np.float32)
    boundary_mask = np.asarray(boundary_mask)
    boundary_prob = np.asarray(boundary_prob, dtype=np.float32)

    p_full = np.clip(boundary_prob[..., -1], np.float32(1e-4), np.float32(1.0 - 1e-4))
    token_idx = np.arange(SEQ)[None, :] + (~boundary_mask).astype(np.int32) * SEQ
    order = np.argsort(token_idx, axis=1, kind="stable")
    gate = np.take_along_axis(p_full, order[:, :MAXC], axis=1)  # [B, C]

    cid = np.cumsum(boundary_mask.astype(np.int32), axis=1) - 1  # [B, S]
    cid = np.clip(cid, 0, MAXC - 1)
    n_used = int(cid.max()) + 1
    NBLK = max(1, math.ceil(n_used / BLK))
    CU = NBLK * BLK

    g = gate[:, :CU].astype(np.float64)
    a = 1.0 - g
    S = np.cumsum(np.log(a), axis=1)  # [B, CU] global log-decay prefix

    # one-block lookback must cover everything older than the previous block
    for t in range(2, NBLK):
        j0 = (t - 1) * BLK - 1
        if np.any(S[:, t * BLK] - S[:, j0] > -18.0):
            raise RuntimeError("128-chunk lookback decay bound violated")

    ii = np.arange(BLK)[:, None]
    jj = np.arange(BLK)[None, :]
    Sb = S.reshape(B, NBLK, BLK)
    # main (within-block) coefficients: L[b,t,i,j] = g_j exp(S_i - S_j), i>=j
    Lf = np.where(
        ii[None, None] >= jj[None, None],
        np.exp(Sb[:, :, :, None] - Sb[:, :, None, :])
        * g.reshape(B, NBLK, 1, BLK),
        0.0,
    )
    # lhsT layout: lt[b, j, t*128 + i]
    LT_sb = np.ascontiguousarray(
        Lf.transpose(0, 3, 1, 2).reshape(B, BLK, NBLK * BLK).astype(np.float16)
    )

    # full-block lookback: chunk (t-1)*128+j feeding out chunk t*128+i
    lt2_sb = np.zeros((B, BLK, NBLK * BLK), dtype=np.float16)
    for t in range(1, NBLK):
        Sout = S[:, t * BLK:(t + 1) * BLK]  # [B, 128]
        Sin = S[:, (t - 1) * BLK:t * BLK]  # [B, 128]
        gin = g[:, (t - 1) * BLK:t * BLK]
        Lb = np.exp(Sout[:, None, :] - Sin[:, :, None]) * gin[:, :, None]
        lt2_sb[:, :, t * BLK:(t + 1) * BLK] = Lb.astype(np.float16)

    # dechunk windows: per token block one 64-aligned 128-chunk window when
    # the union (all cores) span allows it, else aligned 128-blocks.
    cidr = cid.reshape(B, NTB, BLK)
    lo = cidr[:, :, 0].min(axis=0)
    hi = cidr[:, :, -1].max(axis=0)
    wins = []
    chosen = set()
    for tb in range(NTB):
        l, h = int(lo[tb]), int(hi[tb])
        w_hi = (l // 64) * 64          # largest feasible start
        w_lo = max(0, -(-(h - 127) // 64) * 64)  # smallest feasible start
        pick = None
        if w_lo <= w_hi:
            cands = range(w_lo, w_hi + 1, 64)
            for w in cands:  # reuse an already-stitched window if possible
                if w in chosen:
                    pick = w
                    break
            if pick is None:
                for align in (128, 64):
                    for w in cands:
                        if w % align == 0:
                            pick = w
                            break
                    if pick is not None:
                        break
        if pick is None:
            t0, t1 = l // BLK, h // BLK
            wins.append([BLK * t for t in range(t0, t1 + 1)])
        else:
            chosen.add(pick)
            wins.append([pick])
    prod_ws = sorted({w for ws in wins for w in ws if w % BLK != 0})

    # host-precomputed one-hot selection matrices, in emit (tb, window)
    # column order: sel[p, j] = 1 iff cid[tb*128+j] == w + p
    ncols = sum(len(ws) for ws in wins)
    selh = np.zeros((B, BLK, ncols * BLK),
                    dtype=mybir.dt.np(mybir.dt.float8e4))
    col = 0
    for tb in range(NTB):
        for w in wins[tb]:
            c = cid[:, tb * BLK:(tb + 1) * BLK] - w  # [B, 128]
            m = (c >= 0) & (c < BLK)
            bb, jj2 = np.nonzero(m)
            selh[bb, c[bb, jj2], col * BLK + jj2] = 1.0
            col += 1

    in_maps = []
    for b in range(B):
        in_maps.append(
            {
                "x": np.ascontiguousarray(chunk_states[b, :CU].astype(np.float16)),
                "lt": LT_sb[b],
                "lt2": np.ascontiguousarray(lt2_sb[b]),
                "selh": selh[b],
            }
        )
    return in_maps, NBLK, wins, prod_ws


def _build_nc(NBLK, wins, prod_ws):
    ncols = sum(len(ws) for ws in wins)
    nc = bacc.Bacc("TRN2", target_bir_lowering=False, debug=False, num_devices=8)
    x = nc.dram_tensor("x", [NBLK * BLK, DIM], F16, kind="ExternalInput")
    lt = nc.dram_tensor("lt", [BLK, NBLK * BLK], F16, kind="ExternalInput")
    lt2 = nc.dram_tensor("lt2", [BLK, NBLK * BLK], F16, kind="ExternalInput")
    selh = nc.dram_tensor("selh", [BLK, ncols * BLK], F8, kind="ExternalInput")
    out = nc.dram_tensor("out", [SEQ, DIM], F16, kind="ExternalOutput")

    # per-window production step: w -> scan block whose CAST unblocks it
    def wneed(w):
        return w // BLK if w % BLK == 0 else min(w // BLK + 1, NBLK - 1)

    need_zblk = any(w // BLK == NBLK - 1 and w % BLK for w in prod_ws)
    prod_at = {}  # scan block t -> list of windows to stitch after its CAST
    for w in prod_ws:
        prod_at.setdefault(wneed(w), []).append(w)

    # sel DMA split points (in col units), aligned to tb boundaries so each
    # emit group's columns are covered by a whole DMA
    colofs = [0]
    for ws in wins:
        colofs.append(colofs[-1] + len(ws))  # colofs[tb] = first col of tb
    scuts = sorted({colofs[min(4, NTB)], colofs[min(12, NTB)],
                    colofs[min(22, NTB)], ncols})

    with tile.TileContext(nc) as tc:
        with (
            tc.tile_pool(name="const", bufs=1) as const_pool,
            tc.tile_pool(name="outp", bufs=6) as outpool,
            tc.tile_pool(name="ps_scan", bufs=1, space="PSUM") as ps_scan,
            tc.tile_pool(name="ps_out", bufs=3, space="PSUM") as ps_out,
        ):
            # load order: scan weights + x first (critical path) on the sync
            # ring; lookback weights + selection matrices on the scalar ring.
            lt_sb = const_pool.tile([BLK, NBLK * BLK], F16, tag="lt")
            nc.sync.dma_start(lt_sb[:], lt[:])
            x_sb = const_pool.tile([BLK, NBLK * DIM], F16, tag="x")
            # stage x in pieces so early scan blocks unblock fast
            xcuts = sorted({min(1, NBLK), min(2, NBLK), min(3, NBLK),
                            min(5, NBLK), min(7, NBLK), NBLK})
            c_prev = 0
            for c1 in xcuts:
                if c1 <= c_prev:
                    continue
                nc.sync.dma_start(
                    x_sb[:, c_prev * DIM:c1 * DIM],
                    x[c_prev * BLK:c1 * BLK, :].rearrange(
                        "(t p) d -> p t d", p=BLK),
                )
                c_prev = c1
            lt2_sb = const_pool.tile([BLK, NBLK * BLK], F16, tag="lt2")
            nc.scalar.dma_start(lt2_sb[:], lt2[:])
            sel_sb = const_pool.tile([BLK, ncols * BLK], F8, tag="selh")
            c_prev = 0
            for c1 in scuts:
                if c1 <= c_prev:
                    continue
                nc.scalar.dma_start(
                    sel_sb[:, c_prev * BLK:c1 * BLK],
                    selh[:, c_prev * BLK:c1 * BLK],
                )
                c_prev = c1

            # ema: one extra zeroed block so window stitches that read past
            # the last scan block pull zeros (sel never selects them)
            ema = const_pool.tile([BLK, (NBLK + 1) * DIM], F16, tag="ema")
            if need_zblk:
                nc.vector.memset(ema[:, NBLK * DIM:], 0.0)
            win_sb = {}
            for w in prod_ws:
                win_sb[w] = const_pool.tile([BLK, DIM], F16, tag=f"win{w}",
                                            name=f"win_{w}")

            # PE warmup: zero-weight matmuls accumulating into block 0's
            # psum (add 0, cannot be dead-code-eliminated). ~3us of PE
            # activity releases the HAM clock throttle before real work.
            zw = const_pool.tile([BLK, BLK], F16, tag="zw")
            nc.vector.memset(zw[:], 0.0)
            zx = const_pool.tile([BLK, 512], F16, tag="zx")
            nc.vector.memset(zx[:], 0.0)
            ps0 = ps_scan.tile([BLK, DIM], F32, tag="ps")
            for k in range(2):
                for h in range(2):
                    nc.tensor.matmul(
                        ps0[:, h * 512:(h + 1) * 512], lhsT=zw[:], rhs=zx[:],
                        start=(k == 0), stop=False,
                    )

            # ---- dechunk emitter (interleaved with the scan so the PE
            # queue never stalls behind scan blocks waiting on late DMAs) ---
            state = {"tb": 0, "gi": 0, "col": 0}

            def emit_group(grp):
                gi = state["gi"]
                og = outpool.tile([BLK, grp * DIM], F16, tag=f"og{grp}",
                                  name=f"og_{gi}")
                for i in range(grp):
                    tb = state["tb"]
                    ws = wins[tb]
                    po = ps_out.tile([BLK, DIM], F32, tag="po",
                                     name=f"po_{tb}")
                    for wi, w in enumerate(ws):
                        col = state["col"]
                        for h in range(2):
                            if w % BLK == 0:
                                rsrc = ema[:, (w // BLK) * DIM + h * 512:
                                           (w // BLK) * DIM + (h + 1) * 512]
                            else:
                                rsrc = win_sb[w][:, h * 512:(h + 1) * 512]
                            nc.tensor.matmul(
                                po[:, h * 512:(h + 1) * 512],
                                lhsT=sel_sb[:, col * BLK:(col + 1) * BLK],
                                rhs=rsrc,
                                start=(wi == 0),
                                stop=(wi == len(ws) - 1),
                            )
                        state["col"] = col + 1
                    dst = og[:, i * DIM:(i + 1) * DIM]
                    # split the PSUM->SBUF cast across both engines: halves
                    # run concurrently, so the po buffer frees quickly
                    nc.vector.tensor_copy(out=dst[:, :512], in_=po[:, :512])
                    nc.scalar.copy(out=dst[:, 512:], in_=po[:, 512:])
                    state["tb"] = tb + 1
                tb0 = state["tb"] - grp
                dma_eng = nc.scalar if (gi % 2) == 0 else nc.sync
                dma_eng.dma_start(
                    out[tb0 * BLK:state["tb"] * BLK, :].rearrange(
                        "(i p) d -> p i d", p=BLK
                    ),
                    og[:].rearrange("p (i d) -> p i d", d=DIM),
                )
                state["gi"] = gi + 1

            # a group is ready once the last scan block it depends on (via
            # direct windows or stitched ones) is written
            group_need = []
            tb = 0
            for grp in GRPS:
                group_need.append(
                    max(wneed(w) for t in range(tb, tb + grp)
                        for w in wins[t]))
                tb += grp

            # ---- blocked matmul scan over chunk blocks ----
            for t in range(NBLK):
                ps = ps0 if t == 0 else ps_scan.tile([BLK, DIM], F32, tag="ps")
                for h in range(2):
                    sl = slice(h * 512, (h + 1) * 512)
                    xsl = slice(t * DIM + h * 512, t * DIM + (h + 1) * 512)
                    nc.tensor.matmul(
                        ps[:, sl],
                        lhsT=lt_sb[:, t * BLK:(t + 1) * BLK],
                        rhs=x_sb[:, xsl],
                        start=(t != 0),
                        stop=(t == 0),
                    )
                    if t > 0:
                        lsl = slice((t - 1) * DIM + h * 512,
                                    (t - 1) * DIM + (h + 1) * 512)
                        nc.tensor.matmul(
                            ps[:, sl],
                            lhsT=lt2_sb[:, t * BLK:(t + 1) * BLK],
                            rhs=x_sb[:, lsl],
                            start=False,
                            stop=True,
                        )
                # psum -> fp16 ema, split across DVE and ACT
                nc.vector.tensor_copy(
                    out=ema[:, t * DIM:t * DIM + 512], in_=ps[:, :512]
                )
                nc.scalar.copy(
                    out=ema[:, t * DIM + 512:(t + 1) * DIM], in_=ps[:, 512:]
                )
                # stitch any 64-offset windows unblocked by this block's
                # CAST: two legal 64-partition f16 copies (fast on DVE/ACT;
                # GpSimd tensor ops cost ~3.6us each, do not use it)
                for w in prod_at.get(t, ()):
                    t0 = w // BLK
                    nc.vector.tensor_copy(
                        out=win_sb[w][0:64, :],
                        in_=ema[64:BLK, t0 * DIM:(t0 + 1) * DIM],
                    )
                    nc.vector.tensor_copy(
                        out=win_sb[w][64:BLK, :],
                        in_=ema[0:64, (t0 + 1) * DIM:(t0 + 2) * DIM],
                    )
                while (state["gi"] < len(GRPS)
                       and group_need[state["gi"]] <= t):
                    emit_group(GRPS[state["gi"]])

            while state["gi"] < len(GRPS):
                emit_group(GRPS[state["gi"]])

    nc.finalize()
    return nc


def _run(in_maps, NBLK, wins, prod_ws):
    nc = _build_nc(NBLK, wins, prod_ws)
    res = run_bass_kernel_spmd(nc, in_maps, core_ids=list(range(NCORES)))
    return np.stack(
        [res.results[i]["out"].astype(np.float32) for i in range(NCORES)], axis=0
    )


def kernel(chunk_states, boundary_mask, boundary_prob):
    in_maps, NBLK, wins, prod_ws = _preprocess(
        chunk_states, boundary_mask, boundary_prob
    )
    last_err = None
    for _ in range(3):  # retry transient accelerator failures
        try:
            return _run(in_maps, NBLK, wins, prod_ws)
        except Exception as e:  # noqa: BLE001
            last_err = e
            try:
                import jax

                jax.clear_caches()
            except Exception:  # noqa: BLE001
                pass
    raise last_err
